# revision 1
# baseline (speedup 1.0000x reference)
"""MoE expert-collection grouped GEMM for Trainium2, expert-parallel over 8
NeuronCores.

Problem (hardcoded shapes):
  sorted_features  [65536, 1024] f32   tokens sorted by expert, 4096/expert
  expert_ids_sorted[65536] i32         unused: split is static equal-count
  routing_matrix   [1024, 2048, 16] f32
  routing_bias     [2048, 16] f32
  out = silu(x_e @ W_e + b_e) per expert  -> [65536, 2048] f32

Sharding: expert-parallel, 2 experts (= 8192 contiguous sorted tokens) per
core. Host-side "dispatch" hands each core its token block transposed
(feature-major) so the contraction dim lands on SBUF partitions, its 2
experts' weights [2, 1024, 2048], and the per-expert bias pre-broadcast to
128 partitions. Device: fp16 matmul (fp32 PSUM accumulation), DVE bias add,
ACT Silu, fp32 everywhere else.
"""

import numpy as np

import concourse.bass as bass
import concourse.mybir as mybir
import concourse.tile as tile
from concourse.bass_utils import run_bass_kernel_spmd

N_CORES = 8
N_TOKENS = 65536
D_IN = 1024
D_OUT = 2048
N_EXPERTS = 16
E_PER_CORE = N_EXPERTS // N_CORES        # 2
TOK_PER_CORE = N_TOKENS // N_CORES       # 8192
TOK_PER_EXPERT = N_TOKENS // N_EXPERTS   # 4096

P = 128
KB = D_IN // P            # 8 contraction blocks
TS = 512                  # token stripe
OB = 512                  # out-feature block (one PSUM bank)
N_OB = D_OUT // OB        # 4
N_TSUB = TS // P          # 4
STRIPES_PER_EXPERT = TOK_PER_EXPERT // TS  # 8

F32 = mybir.dt.float32
F16 = mybir.dt.float16


def _split_multi_waits(nc):
    """This container's walrus encodes at most ONE sync-wait per instruction;
    hoist extras onto single-wait NoOps inserted just before, same engine."""
    for fn in nc.m.functions:
        for bb in fn.blocks:
            insts = list(bb.instructions)
            out = []
            dirty = False
            for inst in insts:
                si = inst.sync_info
                waits = list(si.on_wait) if si and si.on_wait else []
                if len(waits) > 1:
                    dirty = True
                    for j, w in enumerate(waits[:-1]):
                        nop = mybir.InstNoOp(
                            name=f"{inst.name}-prewait{j}", ins=[], outs=[]
                        )
                        nop.engine = inst.engine
                        nop.sync_info = mybir.SyncInfo(on_wait=[w], on_update=[])
                        out.append(nop)
                    inst.sync_info = mybir.SyncInfo(
                        on_wait=[waits[-1]],
                        on_update=list(si.on_update) if si.on_update else [],
                    )
                out.append(inst)
            if dirty:
                bb.instructions = out


def build_kernel():
    nc = bass.Bass()
    xt = nc.dram_tensor("xt", [D_IN, TOK_PER_CORE], F32, kind="ExternalInput")
    w = nc.dram_tensor("w", [E_PER_CORE, D_IN, D_OUT], F32, kind="ExternalInput")
    bb = nc.dram_tensor("bb", [E_PER_CORE, P, D_OUT], F32, kind="ExternalInput")
    y = nc.dram_tensor("y", [TOK_PER_CORE, D_OUT], F32, kind="ExternalOutput")

    xt_blk = xt.rearrange("(kb p) t -> p kb t", p=P)   # [128, KB, 8192]

    with tile.TileContext(nc) as tc:
        with (
            tc.tile_pool(name="persist", bufs=1) as persist,
            tc.tile_pool(name="wstage", bufs=2) as wstage,
            tc.tile_pool(name="xstage", bufs=2) as xstage,
            tc.tile_pool(name="x16", bufs=2) as x16p,
            tc.tile_pool(name="outs", bufs=3) as outs,
            tc.tile_pool(name="psum", bufs=6, space="PSUM") as psump,
        ):
            # Resident weights (fp16) + bias (fp32, pre-broadcast on host)
            w16 = persist.tile([P, E_PER_CORE * KB, D_OUT], F16)
            for e in range(E_PER_CORE):
                for kb in range(KB):
                    wst = wstage.tile([P, D_OUT], F32)
                    nc.sync.dma_start(wst[:], w[e, kb * P:(kb + 1) * P, :])
                    nc.vector.tensor_copy(w16[:, e * KB + kb, :], wst[:])
            b_sb = persist.tile([P, E_PER_CORE, D_OUT], F32)
            nc.sync.dma_start(
                b_sb[:], bb.rearrange("e p o -> p e o")
            )

            for e in range(E_PER_CORE):
                for s in range(STRIPES_PER_EXPERT):
                    t0 = e * TOK_PER_EXPERT + s * TS
                    x32 = xstage.tile([P, KB, TS], F32)
                    nc.sync.dma_start(x32[:], xt_blk[:, :, t0:t0 + TS])
                    x16 = x16p.tile([P, KB, TS], F16)
                    nc.vector.tensor_copy(x16[:], x32[:])

                    for tsub in range(N_TSUB):
                        y_act = outs.tile([P, D_OUT], F32)
                        for ob in range(N_OB):
                            ps = psump.tile([P, OB], F32)
                            for kb in range(KB):
                                nc.tensor.matmul(
                                    ps[:],
                                    lhsT=x16[:, kb, tsub * P:(tsub + 1) * P],
                                    rhs=w16[:, e * KB + kb, ob * OB:(ob + 1) * OB],
                                    start=(kb == 0),
                                    stop=(kb == KB - 1),
                                )
                            y_sb = outs.tile([P, OB], F32, tag="ysb")
                            nc.vector.tensor_tensor(
                                y_sb[:], ps[:], b_sb[:, e, ob * OB:(ob + 1) * OB],
                                mybir.AluOpType.add,
                            )
                            nc.scalar.activation(
                                y_act[:, ob * OB:(ob + 1) * OB], y_sb[:],
                                mybir.ActivationFunctionType.Silu,
                            )
                        nc.sync.dma_start(
                            y[t0 + tsub * P:t0 + (tsub + 1) * P, :], y_act[:]
                        )

    _split_multi_waits(nc)
    return nc


_NC_CACHE = None


def _get_nc():
    global _NC_CACHE
    if _NC_CACHE is None:
        _NC_CACHE = build_kernel()
    return _NC_CACHE


def _in_maps(sorted_features, routing_matrix, routing_bias):
    maps = []
    for c in range(N_CORES):
        rows = slice(c * TOK_PER_CORE, (c + 1) * TOK_PER_CORE)
        es = slice(c * E_PER_CORE, (c + 1) * E_PER_CORE)
        xt_c = np.ascontiguousarray(sorted_features[rows].T)
        w_c = np.ascontiguousarray(
            routing_matrix[:, :, es].transpose(2, 0, 1)
        )
        b_c = np.ascontiguousarray(
            np.broadcast_to(
                routing_bias[:, es].T[:, None, :], (E_PER_CORE, P, D_OUT)
            )
        )
        maps.append({"xt": xt_c, "w": w_c, "bb": b_c})
    return maps


def run(sorted_features, routing_matrix, routing_bias, **run_kwargs):
    nc = _get_nc()
    maps = _in_maps(sorted_features, routing_matrix, routing_bias)
    res = run_bass_kernel_spmd(nc, maps, core_ids=list(range(N_CORES)), **run_kwargs)
    out = np.concatenate([res.results[c]["y"] for c in range(N_CORES)], axis=0)
    return out, res


def kernel(sorted_features, expert_ids_sorted, routing_matrix, routing_bias):
    assert sorted_features.shape == (N_TOKENS, D_IN)
    assert routing_matrix.shape == (D_IN, D_OUT, N_EXPERTS)
    assert routing_bias.shape == (D_OUT, N_EXPERTS)
    out, _ = run(
        np.asarray(sorted_features, dtype=np.float32),
        np.asarray(routing_matrix, dtype=np.float32),
        np.asarray(routing_bias, dtype=np.float32),
    )
    return out
